# revision 27
# baseline (speedup 1.0000x reference)
"""Trainium2 Bass kernel for nn_AgnisV5 (B=4, T=256, V=50257, D=768, H=3072).

Strategy
--------
The reference is a 256-step sequential recurrence over h (LayerNorm'd each
step) plus a big lm_head projection that does not feed back. The recurrence
map is contractive, so the whole sequence is solved by batched Picard
sweeps: H <- StepAll(shift(H)), each sweep a full-width (M=128/core) pass
over all timesteps, time-sharded across 8 cores.

The h-dependence of the step map splits into a cheap temporal path
(h @ RWt, gated) and an expensive hierarchy path (V0/V1/W1/W2 MLPs of ctx).
The hierarchy path has low sensitivity to h, so most sweeps FREEZE it:
after a full sweep computes gate G and core_feat CF, store P = G*(CF-E)+E
and Ga = alpha*G; a cheap sweep is then just h <- LN(P + Ga*(shift(h)@RWt)).
Schedule FFCCCCCFCCCCCCCC (2 full + one mid refresh + 13 cheap), lag-3
bf16 halo. CPU-simulated end-to-end rel err 5.6e-3, measured 5.4e-3
(gate 2e-2).

The boundary halo is a per-sweep ReduceScatter: each core masks its edge
into the successor's chunk (host-provided mask), so the receive side is a
single contiguous DMA with no select/blend. LN/l2n are lean: h and h^2
packed in one bf16 tile summed by a single 6-matmul PSUM chain, rsqrt via
the 40k-entry Abs_reciprocal_sqrt ACT table (Newton fallback on the final
sweep), one f32r broadcast matmul, stride-0 broadcast APs for the apply,
and the shifted next-sweep input written directly by the LN apply. The
gate sigmoid is computed as (1+tanh(x/2))/2 so all F-sweep activations
(Gelu + Tanh) share one ACT table set - no per-sweep table reloads.

Startup: weight DMAs are chunked and alternated across both HWDGE queues
(sync/scalar) in first-use order; the PE warms up on embedding data while
they stream. lm_head: vocab-sharded bf16 weights DMA-preloaded during the
cheap tail into SBUF freed by the full-sweep weights; one bf16 AllGather
of final H, per-block gather-in DMAs.
"""
import sys, os
sys.path.insert(0, '/opt/trn_rl_repo')
import numpy as np
import ml_dtypes

import concourse.bass as bass
import concourse.bacc as bacc
import concourse.mybir as mybir
import concourse.tile as tile
from concourse.bass_utils import run_bass_kernel_spmd


def _ensure_ntff_hook():
    """The agent image's antenv lacks axon_hooks, which silently disables
    NTFF profiling (exec_time_ns). Shim the module and register the
    ctypes-based hook from trn_agent_boot if available."""
    import types
    if "antenv.axon_hooks" in sys.modules:
        return
    try:
        import antenv
        m = types.ModuleType("antenv.axon_hooks")
        _h = [None]
        m.set_axon_ntff_profile_hook = lambda h: _h.__setitem__(0, h)
        m.get_axon_ntff_profile_hook = lambda: _h[0]
        sys.modules["antenv.axon_hooks"] = m
        antenv.axon_hooks = m
        from trn_agent_boot.trn_boot import _ntff_profile_via_ctypes
        hook = _ntff_profile_via_ctypes("/opt/axon/libaxon_pjrt.so")
        if hook is not None:
            m.set_axon_ntff_profile_hook(hook)
    except Exception:
        pass


_ensure_ntff_hook()

F32 = mybir.dt.float32
F32R = mybir.dt.float32r
BF16 = mybir.dt.bfloat16
I32 = mybir.dt.int32
AF = mybir.ActivationFunctionType
ALU = mybir.AluOpType

N_CORES = 8
B, T, V, D, H = 4, 256, 50257, 768, 3072
ROWS = 128                 # rows per core = 32 timesteps x 4 batch
KC_D = D // 128            # 6 chunks of the d dimension
KC_H = H // 128            # 24 chunks of the hidden dimension
VPAD = 6400                # per-core vocab shard cols, padded to 50*128
VSHARD = 6283              # ceil(V / 8); host pads vocab to 8*VSHARD = 50264
SCHEDULE = "FFCCCCCFCCCCCCCC"
ALPHA = 0.4

LAST_RESULT = None         # BassKernelResults of the most recent run (for test.py)

_BUILD_CACHE = {}


def _t_layout(w):
    """[K, M] row-major -> [128, K/128, M] T-layout for stationary lhsT tiles."""
    K, M = w.shape
    assert K % 128 == 0
    return np.ascontiguousarray(w.reshape(K // 128, 128, M).transpose(1, 0, 2))


def _bcast_mid(ap2d, n):
    """[128, R] AP -> [128, n, R] stride-0 broadcast along the middle dim."""
    return bass.AP(ap2d.tensor, ap2d.offset, [ap2d.ap[0], (0, n), ap2d.ap[1]])


def build(schedule=SCHEDULE):
    n_sweeps = len(schedule)
    last_f = max(i for i, k in enumerate(schedule) if k == 'F')
    nc = bacc.Bacc("TRN2", target_bir_lowering=False, debug=False,
                   num_devices=N_CORES)

    # ---- DRAM parameters (per-core data via in_maps) ----
    embT_ext = nc.declare_dram_parameter("embT", [128, KC_D, ROWS], F32, isOutput=False)
    hmask_ext = nc.declare_dram_parameter("hmask", [128, 8, KC_D * 4], BF16,
                                          isOutput=False)
    wb_ext = {}
    for name, (wk, wm) in dict(Wgt=(D, D), V0=(D, H), V1=(H, D), W1=(D, D),
                               W2=(D, D), W2Wg=(D, D), RWt=(D, D), R=(D, D)).items():
        wb_ext[name] = nc.declare_dram_parameter(f"wb_{name}", [128, wk // 128, wm],
                                                 BF16, isOutput=False)
    wl_ext = nc.declare_dram_parameter("wl", [VPAD // 128, 128, KC_D, 128], BF16, isOutput=False)
    out_ext = nc.declare_dram_parameter("out", [VPAD, T * B], F32, isOutput=True)
    warm_ext = nc.declare_dram_parameter("warm", [128, 4], F32, isOutput=True)

    # ---- internal DRAM for collectives ----
    halo_in = [nc.dram_tensor(f"halo_in_{k}", [N_CORES * 128, KC_D * 4], BF16)
               for k in range(n_sweeps)]
    halo_out = [nc.dram_tensor(f"halo_out_{k}", [128, KC_D * 4], BF16)
                for k in range(n_sweeps)]
    ccw_in = nc.dram_tensor("ccw_in", [1, 32], F32)
    ccw_out = nc.dram_tensor("ccw_out", [N_CORES, 32], F32, addr_space="Shared")
    hfin_in = nc.dram_tensor("hfin_in", [128, KC_D * ROWS], BF16)
    hfin_out = nc.dram_tensor("hfin_out", [N_CORES * 128, KC_D * ROWS], BF16,
                              addr_space="Shared")

    rg = [list(range(N_CORES))]

    with tile.TileContext(nc) as tc:
        with (
            tc.tile_pool(name="cpool", bufs=1) as cpool,      # persistents
            tc.tile_pool(name="apool", bufs=1) as apool,      # activations
            tc.tile_pool(name="npool", bufs=1) as npool,      # norm scratch
            tc.tile_pool(name="pps", bufs=4, space="PSUM") as pps,
            tc.tile_pool(name="sps", bufs=2, space="PSUM") as sps,
        ):
            # ---------- persistent loads ----------
            embT = cpool.tile([128, KC_D, ROWS], F32, tag="embT")
            nc.sync.dma_start(embT[:], embT_ext[:])
            hmask = cpool.tile([128, 8, KC_D * 4], BF16, tag="hmask")
            nc.scalar.dma_start(hmask[:], hmask_ext[:])
            embTbf = cpool.tile([128, KC_D, ROWS], BF16, tag="embTbf")
            nc.vector.tensor_copy(embTbf[:], embT[:])
            rwt_t = cpool.tile([128, KC_D, D], BF16, tag="w_RWt")
            # warm up the collective path early (first call pays ENCD init)
            nc.sync.dma_start(ccw_in[:], embT[0:1, 0, 0:32])
            nc.gpsimd.collective_compute(
                "AllGather", ALU.bypass, replica_groups=rg,
                ins=[ccw_in[:]], outs=[ccw_out[:]])

            ones_col_bf = cpool.tile([128, 1], BF16, tag="ones_col_bf")
            nc.vector.memset(ones_col_bf[:], 1.0)
            ones_row_f = cpool.tile([1, 128], F32, tag="ones_row_f")
            nc.vector.memset(ones_row_f[:], 1.0)
            ones_row_r = cpool.tile([1, 128], F32R, tag="ones_row_r")
            nc.vector.tensor_copy(ones_row_r[:], ones_row_f[:])

            # persistent state
            Hs = [cpool.tile([128, KC_D, ROWS], BF16, tag=f"Hs{i}", name=f"Hs{i}")
                  for i in range(2)]
            EG = cpool.tile([128, KC_D, ROWS], F32, tag="EG")
            Psb = cpool.tile([128, KC_D, ROWS], F32, tag="Psb")    # frozen G*(CF-E)+E
            Gab = cpool.tile([128, KC_D, ROWS], BF16, tag="Gab")   # frozen alpha*G
            Hfbf = cpool.tile([128, KC_D, ROWS], BF16, tag="Hfbf")  # final H

            def mm_layer(w, Kc, Mc, rhs_fn, consume, group=4):
                wap = w if callable(w) else (
                    lambda kc, mc: w[:, kc, mc * 128:(mc + 1) * 128])
                for m0 in range(0, Mc, group):
                    g = min(group, Mc - m0)
                    p = pps.tile([128, g * 128], F32, tag="mmps")
                    for sub in range(g):
                        mc = m0 + sub
                        for kc in range(Kc):
                            nc.tensor.matmul(
                                p[:, sub * 128:(sub + 1) * 128],
                                wap(kc, mc),
                                rhs_fn(kc),
                                start=(kc == 0), stop=(kc == Kc - 1))
                    consume(p, m0, g)

            def nr_rsqrt_into(out_ap, s_ap, n, name, iters):
                """out_ap[1, n] = rsqrt(s_ap) via bit-trick seed + Newton."""
                bits = npool.tile([1, n], I32, tag=f"{name}b")
                nc.vector.tensor_scalar(bits[:], s_ap.bitcast(I32), 1, None,
                                        ALU.logical_shift_right)
                nc.vector.tensor_scalar(bits[:], bits[:], -1, 0x5f3759df,
                                        ALU.mult, ALU.add)
                cur = bits[:].bitcast(F32)
                for i in range(iters):
                    w = npool.tile([1, n], F32, tag=f"{name}w")
                    nc.vector.tensor_tensor(w[:], cur, cur, ALU.mult)
                    nc.vector.tensor_tensor(w[:], w[:], s_ap, ALU.mult)
                    nc.vector.tensor_scalar(w[:], w[:], -0.5, 1.5, ALU.mult, ALU.add)
                    if i == iters - 1:
                        nc.vector.tensor_tensor(out_ap, cur, w[:], ALU.mult)
                    else:
                        y = npool.tile([1, n], F32, tag=f"{name}y{i}")
                        nc.vector.tensor_tensor(y[:], cur, w[:], ALU.mult)
                        cur = y[:]

            def layer_norm_apply(hb2, it, nxt):
                """LN over hb2[:,:,0,:]; writes shifted nxt (or Hfbf if final)."""
                final = (it == n_sweeps - 1)
                nc.vector.tensor_tensor(hb2[:, 1], hb2[:, 0],
                                        hb2[:, 0], ALU.mult)
                ssum = sps.tile([1, 2 * ROWS], F32, tag="sum_ps")
                for kc in range(KC_D):
                    nc.tensor.matmul(ssum[:], ones_col_bf[:], hb2[:, :, kc, :],
                                     start=(kc == 0), stop=(kc == KC_D - 1))
                bc_in = npool.tile([1, 2 * ROWS], F32, tag="bc_in")
                nc.vector.tensor_scalar(bc_in[:, 0:ROWS], ssum[:, 0:ROWS],
                                        1.0 / D, None, ALU.mult)
                msq = npool.tile([1, ROWS], F32, tag="msq")
                nc.vector.tensor_tensor(msq[:], bc_in[:, 0:ROWS], bc_in[:, 0:ROWS],
                                        ALU.mult)
                var = npool.tile([1, ROWS], F32, tag="var")
                nc.vector.scalar_tensor_tensor(var[:], ssum[:, ROWS:2 * ROWS],
                                               1.0 / D, msq[:], ALU.mult,
                                               ALU.subtract)
                nc.vector.tensor_scalar(var[:], var[:], 1e-5, None, ALU.add)
                if final:
                    nr_rsqrt_into(bc_in[:, ROWS:2 * ROWS], var[:], ROWS, "ln",
                                  iters=2)
                else:
                    nc.scalar.activation(bc_in[:, ROWS:2 * ROWS], var[:],
                                         AF.Abs_reciprocal_sqrt)
                bcr = npool.tile([1, 2 * ROWS], F32R, tag="bcr")
                nc.vector.tensor_copy(bcr[:], bc_in[:])
                brp = pps.tile([128, 2 * ROWS], F32, tag="brp", bufs=2)
                nc.tensor.matmul(brp[:], ones_row_r[:], bcr[:], start=True, stop=True)
                dt_ = npool.tile([128, KC_D, ROWS], BF16, tag="dt")
                nc.vector.tensor_tensor(dt_[:], hb2[:, 0],
                                        _bcast_mid(brp[:, 0:ROWS], KC_D),
                                        ALU.subtract)
                if final:
                    nc.vector.tensor_tensor(Hfbf[:], dt_[:],
                                            _bcast_mid(brp[:, ROWS:2 * ROWS], KC_D),
                                            ALU.mult)
                    return None
                # shifted store: nxt rows 4.. <- own rows 0..123
                nc.vector.tensor_tensor(
                    nxt[:, :, 4:ROWS], dt_[:, :, 0:ROWS - 4],
                    _bcast_mid(brp[:, ROWS:2 * ROWS - 4], KC_D), ALU.mult)
                edge = apool.tile([128, KC_D, 4], BF16, tag="edge")
                nc.vector.tensor_tensor(
                    edge[:], dt_[:, :, ROWS - 4:ROWS],
                    _bcast_mid(brp[:, 2 * ROWS - 4:2 * ROWS], KC_D), ALU.mult)
                return edge

            def sweep_tail(it, nxt, edge):
                """Launch this sweep's halo; consume the lag-3 halo into nxt.

                The halo is a ReduceScatter: each core contributes its edge
                masked into the successor core's chunk (hmask, host data), so
                the receive side is a single contiguous DMA with no blend."""
                if it <= n_sweeps - 3:
                    masked = npool.tile([128, 8, KC_D * 4], BF16, tag="masked")
                    eflat = edge[:].rearrange("p k c -> p (k c)")
                    nc.vector.tensor_tensor(masked[:], _bcast_mid(eflat, 8),
                                            hmask[:], ALU.mult)
                    half_rows = 4 * 128
                    nc.sync.dma_start(
                        halo_in[it].ap()[0:half_rows, :].rearrange(
                            "(j p) f -> p j f", p=128),
                        masked[:, 0:4, :])
                    nc.scalar.dma_start(
                        halo_in[it].ap()[half_rows:2 * half_rows, :].rearrange(
                            "(j p) f -> p j f", p=128),
                        masked[:, 4:8, :])
                    nc.gpsimd.collective_compute(
                        "ReduceScatter", ALU.add, replica_groups=rg,
                        ins=[halo_in[it][:]], outs=[halo_out[it][:]])
                if it >= 2:
                    hstage = npool.tile([128, KC_D * 4], BF16, tag="hstage")
                    nc.sync.dma_start(hstage[:], halo_out[it - 2][:])
                    nc.vector.tensor_copy(
                        nxt[:, :, 0:4],
                        hstage[:].rearrange("p (k c) -> p k c", k=KC_D))
                else:
                    nc.vector.memset(nxt[:, :, 0:4], 0.0)

            def hp_consume_fn(hb2):
                def f(p, m0, g):
                    t2 = apool.tile([128, g * 128], F32, tag=f"t2_{m0 % 8}")
                    nc.vector.tensor_tensor(t2[:], p[:], Gab[:, m0:m0 + g, :],
                                            ALU.mult)
                    nc.vector.tensor_tensor(hb2[:, 0, m0:m0 + g, :], t2[:],
                                            Psb[:, m0:m0 + g, :], ALU.add)
                return f

            def full_sweep(it, wsb, cur, nxt):
                first = (it == 0)
                if first:
                    CTX = embTbf
                else:
                    CTX = apool.tile([128, KC_D, ROWS], BF16, tag="CTX", bufs=2)

                    def ctx_consume(p, m0, g):
                        nc.vector.scalar_tensor_tensor(
                            CTX[:, m0:m0 + g, :], p[:], ALPHA,
                            embT[:, m0:m0 + g, :], ALU.mult, ALU.add)
                    mm_layer(wsb["R"], KC_D, KC_D, lambda kc: cur[:, kc, :], ctx_consume)

                Abf = apool.tile([128, KC_H, ROWS], BF16, tag="Abf")

                def gelu_consume(dst):
                    def f(p, m0, g):
                        nc.scalar.activation(dst[:, m0:m0 + g, :], p[:], AF.Gelu)
                    return f
                mm_layer(wsb["V0"], KC_D, KC_H, lambda kc: CTX[:, kc, :],
                         gelu_consume(Abf))
                TGTbf = apool.tile([128, KC_D, ROWS], BF16, tag="TGTbf", bufs=2)
                mm_layer(wsb["V1"], KC_H, KC_D, lambda kc: Abf[:, kc, :],
                         gelu_consume(TGTbf))

                # TF matmuls early: fill the PE gap while the l2n chain runs
                tf_ps = []
                if not first:
                    mm_layer(rwt_t, KC_D, KC_D, lambda kc: cur[:, kc, :],
                             lambda p, m0, g: tf_ps.append((p, m0, g)))

                # CB = l2n(TGT)
                sq = npool.tile([128, KC_D, ROWS], BF16, tag="sq")
                nc.vector.tensor_tensor(sq[:], TGTbf[:], TGTbf[:], ALU.mult)
                ssp = sps.tile([1, 2 * ROWS], F32, tag="sum_ps")
                for kc in range(KC_D):
                    nc.tensor.matmul(ssp[:, 0:ROWS], ones_col_bf[:], sq[:, kc, :],
                                     start=(kc == 0), stop=(kc == KC_D - 1))
                ss = npool.tile([1, ROWS], F32, tag="ss")
                nc.vector.tensor_scalar(ss[:], ssp[:, 0:ROWS], 1e-24, None, ALU.add)
                rl2 = npool.tile([1, ROWS], F32, tag="rl2")
                nr_rsqrt_into(rl2[:], ss[:], ROWS, "l2n", iters=2)
                rl2r = npool.tile([1, ROWS], F32R, tag="rl2r")
                nc.vector.tensor_copy(rl2r[:], rl2[:])
                rbp = pps.tile([128, ROWS], F32, tag="brp", bufs=2)
                nc.tensor.matmul(rbp[:], ones_row_r[:], rl2r[:], start=True, stop=True)
                CBbf = apool.tile([128, KC_D, ROWS], BF16, tag="CBbf")
                nc.vector.tensor_tensor(CBbf[:], TGTbf[:], _bcast_mid(rbp[:], KC_D),
                                        ALU.mult)

                Ubf = apool.tile([128, KC_D, ROWS], BF16, tag="Ubf", bufs=2)
                mm_layer(wsb["W1"], KC_D, KC_D, lambda kc: CBbf[:, kc, :],
                         gelu_consume(Ubf))

                if first:
                    # EG = embT @ Wg_top, emitted here so its matmuls sit
                    # behind F0's V0/V1 work in PE program order (Wgt's DMA
                    # arrives later than V0's).
                    def eg_consume(p, m0, g):
                        nc.vector.tensor_copy(EG[:, m0:m0 + g, :], p[:])
                    mm_layer(wsb["Wgt"], KC_D, KC_D, lambda kc: embTbf[:, kc, :],
                             eg_consume)

                CFbf = apool.tile([128, KC_D, ROWS], BF16, tag="CFbf", bufs=2)

                def cf_consume(p, m0, g):
                    nc.scalar.copy(CFbf[:, m0:m0 + g, :], p[:])
                mm_layer(wsb["W2"], KC_D, KC_D, lambda kc: Ubf[:, kc, :], cf_consume)

                # gate via tanh (shares the Gelu ACT table):
                # G = sigmoid(x) = (1+tanh(x/2))/2; store th = tanh(x/2)
                Gsb = apool.tile([128, KC_D, ROWS], BF16, tag="Gsb")

                def g_consume(p, m0, g):
                    gin = apool.tile([128, g * 128], F32, tag=f"gin{m0 % 8}")
                    nc.vector.tensor_tensor(gin[:], p[:], EG[:, m0:m0 + g, :], ALU.add)
                    nc.scalar.activation(Gsb[:, m0:m0 + g, :], gin[:], AF.Tanh,
                                         scale=0.5)
                mm_layer(wsb["W2Wg"], KC_D, KC_D, lambda kc: Ubf[:, kc, :], g_consume)

                # Ga = alpha*(1+th)/2 ; P = ((1+th)/2)*(CF-E)+E
                nc.vector.tensor_scalar(Gab[:], Gsb[:], ALPHA / 2, ALPHA / 2,
                                        ALU.mult, ALU.add)
                t_ = apool.tile([128, KC_D, ROWS], F32, tag="pt")
                nc.vector.tensor_tensor(t_[:], CFbf[:], embT[:], ALU.subtract)
                nc.vector.scalar_tensor_tensor(t_[:], Gsb[:], 1.0, t_[:],
                                               ALU.add, ALU.mult)
                nc.vector.scalar_tensor_tensor(Psb[:], t_[:], 0.5, embT[:],
                                               ALU.mult, ALU.add)

                hb2 = npool.tile([128, 2, KC_D, ROWS], BF16, tag="hb2")
                if first:
                    nc.vector.tensor_copy(hb2[:, 0], Psb[:])
                else:
                    hpc = hp_consume_fn(hb2)
                    for (p, m0, g) in tf_ps:
                        hpc(p, m0, g)
                return layer_norm_apply(hb2, it, nxt)

            def cheap_sweep(it, cur, nxt):
                hb2 = npool.tile([128, 2, KC_D, ROWS], BF16, tag="hb2")
                mm_layer(rwt_t, KC_D, KC_D, lambda kc: cur[:, kc, :],
                         hp_consume_fn(hb2))
                return layer_norm_apply(hb2, it, nxt)

            # ---------- phase 1: sweeps up to and including the last F ----------
            with tc.tile_pool(name="fwpool", bufs=1) as fwpool:
                # PE warm-up on data available immediately: ramps the PE clock
                # while the weight DMAs stream in.
                wu_ps = pps.tile([128, 512], F32, tag="mmps")
                for i in range(12):
                    nc.tensor.matmul(wu_ps[:], embTbf[:, 0, 0:128],
                                     embTbf[:, (i % 3):(i % 3) + 4, :],
                                     start=(i == 0), stop=(i == 11))
                wu_sb = cpool.tile([128, 4], F32, tag="wu_sb")
                nc.vector.tensor_copy(wu_sb[:], wu_ps[:, 0:4])
                nc.sync.dma_start(warm_ext[:], wu_sb[:])

                # weight loads: chunked and alternated across the two DMA
                # engines, ordered by first use so F0 can start immediately.
                qi = [0]

                def qdma(dst_ap, src_ap):
                    eng = nc.sync if qi[0] % 2 == 0 else nc.scalar
                    qi[0] += 1
                    eng.dma_start(dst_ap, src_ap)

                v0_t = [fwpool.tile([128, H], BF16, tag=f"w_V0_{kc}", name=f"w_V0_{kc}")
                        for kc in range(KC_D)]
                for kc in range(KC_D):
                    qdma(v0_t[kc][:], wb_ext["V0"][:, kc, :])
                v1_t = [fwpool.tile([128, 6, D], BF16, tag=f"w_V1_{b}", name=f"w_V1_{b}")
                        for b in range(4)]
                for b_ in range(4):
                    qdma(v1_t[b_][:], wb_ext["V1"][:, b_ * 6:(b_ + 1) * 6, :])
                wsb = {"V0": lambda kc, mc: v0_t[kc][:, mc * 128:(mc + 1) * 128],
                       "V1": lambda kc, mc: v1_t[kc // 6][:, kc % 6,
                                                          mc * 128:(mc + 1) * 128]}
                for name in ("W1", "W2", "W2Wg", "Wgt", "R"):
                    ext = wb_ext[name]
                    t_ = fwpool.tile(list(ext.shape), BF16, tag=f"w_{name}")
                    qdma(t_[:], ext[:])
                    wsb[name] = t_
                qdma(rwt_t[:], wb_ext["RWt"][:])

                for it in range(last_f + 1):
                    cur = Hs[it % 2]
                    nxt = Hs[(it + 1) % 2]
                    if schedule[it] == 'F':
                        edge = full_sweep(it, wsb, cur, nxt)
                    else:
                        edge = cheap_sweep(it, cur, nxt)
                    sweep_tail(it, nxt, edge)

            # ---------- phase 2: cheap tail + lm_head ----------
            with (
                tc.tile_pool(name="wlpool", bufs=1) as wlpool,
                tc.tile_pool(name="opool", bufs=4) as opool,
            ):
                NV = VPAD // 128
                wl_t = wlpool.tile([128, NV, KC_D, 128], BF16, tag="wl")
                for vc in range(NV):
                    nc.scalar.dma_start(wl_t[:, vc], wl_ext[vc])

                for it in range(last_f + 1, n_sweeps):
                    cur = Hs[it % 2]
                    nxt = Hs[(it + 1) % 2]
                    edge = cheap_sweep(it, cur, nxt)
                    if it + 1 < n_sweeps:
                        sweep_tail(it, nxt, edge)

                # final H gather (bf16)
                nc.sync.dma_start(
                    hfin_in.ap().rearrange("p (k c) -> p k c", k=KC_D), Hfbf[:])
                nc.gpsimd.collective_compute(
                    "AllGather", ALU.bypass, replica_groups=rg,
                    ins=[hfin_in[:]], outs=[hfin_out[:]])
                Hfull = cpool.tile([128, KC_D, T * B], BF16, tag="Hfull")
                for r in range(N_CORES):
                    eng = nc.sync if r % 2 == 0 else nc.scalar
                    eng.dma_start(
                        Hfull[:, :, r * 128:(r + 1) * 128],
                        hfin_out.ap()[r * 128:(r + 1) * 128, :].rearrange(
                            "p (k c) -> p k c", k=KC_D))

                for vc in range(NV):
                    for half in range(2):
                        p = pps.tile([128, 512], F32, tag="mmps")
                        for kc in range(KC_D):
                            nc.tensor.matmul(
                                p[:], wl_t[:, vc, kc, :],
                                Hfull[:, kc, half * 512:(half + 1) * 512],
                                start=(kc == 0), stop=(kc == KC_D - 1))
                        osb = opool.tile([128, 512], F32, tag="osb")
                        if half == 0:
                            nc.vector.tensor_copy(osb[:], p[:])
                        else:
                            nc.scalar.copy(osb[:], p[:])
                        nc.sync.dma_start(
                            out_ext[vc * 128:(vc + 1) * 128,
                                    half * 512:(half + 1) * 512], osb[:])

    nc.compile()
    return nc


def _get_built(schedule=SCHEDULE):
    if schedule not in _BUILD_CACHE:
        _BUILD_CACHE[schedule] = build(schedule)
    return _BUILD_CACHE[schedule]


def _prep_in_maps(token_ids, embedding, V0, b0, V1, b1, W1, c1, W2, c2, Wg, bg,
                  Wt, gamma, beta, Wl, R_weight):
    f64 = np.float64
    for z in (b0, b1, c1, c2, bg, beta):
        assert np.count_nonzero(np.asarray(z)) == 0, "nonzero bias unsupported"
    assert np.allclose(np.asarray(gamma), 1.0), "gamma != 1 unsupported"

    tok = np.asarray(token_ids).astype(np.int64)           # [B, T]
    emb = np.asarray(embedding, f64)[tok]                  # [B, T, D]
    emb = emb / np.maximum(np.linalg.norm(emb, axis=-1, keepdims=True), 1e-12)
    rows = emb.transpose(1, 0, 2).reshape(T * B, D)        # row = t*4+b

    bf = ml_dtypes.bfloat16
    wt = {
        "R": _t_layout(np.asarray(R_weight, f64)).astype(bf),
        "V0": _t_layout(np.asarray(V0, f64)).astype(bf),
        "V1": _t_layout(np.asarray(V1, f64)).astype(bf),
        "W1": _t_layout(np.asarray(W1, f64)).astype(bf),
        "W2": _t_layout(np.asarray(W2, f64)).astype(bf),
        "RWt": _t_layout(np.asarray(R_weight, f64) @ np.asarray(Wt, f64)).astype(bf),
        "Wgt": _t_layout(np.asarray(Wg, f64)[:D]).astype(bf),
        "W2Wg": _t_layout(np.asarray(W2, f64) @ np.asarray(Wg, f64)[D:]).astype(bf),
    }
    wl_f32 = np.asarray(Wl, np.float32)

    in_maps = []
    for c in range(N_CORES):
        block = rows[c * ROWS:(c + 1) * ROWS].T            # [D, 128]
        embT = np.ascontiguousarray(
            block.reshape(KC_D, 128, ROWS).transpose(1, 0, 2)).astype(np.float32)
        hm = np.zeros((8, KC_D * 4), np.float32)
        if c < N_CORES - 1:
            hm[c + 1, :] = 1.0
        hmask = np.broadcast_to(hm, (128, 8, KC_D * 4)).astype(bf)
        wl_shard_cols = np.zeros((D, VPAD), np.float32)
        lo = c * VSHARD
        hi = min(V, lo + VSHARD)
        wl_shard_cols[:, :hi - lo] = wl_f32[:, lo:hi]
        wl_shard = _t_layout(wl_shard_cols)                 # [128, KC_D, VPAD]
        wl_shard = np.ascontiguousarray(
            wl_shard.reshape(128, KC_D, VPAD // 128, 128).transpose(2, 0, 1, 3)).astype(bf)
        m = {"embT": embT, "hmask": np.ascontiguousarray(hmask), "wl": wl_shard}
        for name, w in wt.items():
            m[f"wb_{name}"] = w
        in_maps.append(m)
    return in_maps


def kernel(**inputs):
    global LAST_RESULT
    in_maps = _prep_in_maps(**{k: np.asarray(v) for k, v in inputs.items()})
    nc = _get_built()
    trace = bool(os.environ.get("KERNEL_TRACE"))
    res = run_bass_kernel_spmd(nc, in_maps, core_ids=list(range(N_CORES)),
                               trace=trace)
    LAST_RESULT = res
    parts = [res.results[c]["out"][:VSHARD] for c in range(N_CORES)]
    L = np.concatenate(parts, axis=0)[:V]                  # [V, T*B]
    out = np.ascontiguousarray(
        L.reshape(V, T, B).transpose(2, 1, 0)).astype(np.float32)
    return out


if __name__ == "__main__":
    pass


# revision 28
# speedup vs baseline: 1.0113x; 1.0113x over previous
"""Trainium2 Bass kernel for nn_AgnisV5 (B=4, T=256, V=50257, D=768, H=3072).

Strategy
--------
The reference is a 256-step sequential recurrence over h (LayerNorm'd each
step) plus a big lm_head projection that does not feed back. The recurrence
map is contractive, so the whole sequence is solved by batched Picard
sweeps: H <- StepAll(shift(H)), each sweep a full-width (M=128/core) pass
over all timesteps, time-sharded across 8 cores.

The h-dependence of the step map splits into a cheap temporal path
(h @ RWt, gated) and an expensive hierarchy path (V0/V1/W1/W2 MLPs of ctx).
The hierarchy path has low sensitivity to h, so most sweeps FREEZE it:
after a full sweep computes gate G and core_feat CF, store P = G*(CF-E)+E
and Ga = alpha*G; a cheap sweep is then just h <- LN(P + Ga*(shift(h)@RWt)).
Schedule FFCCCCCFCCCCCCCC (2 full + one mid refresh + 13 cheap), lag-3
bf16 halo. CPU-simulated end-to-end rel err 5.6e-3, measured 5.4e-3
(gate 2e-2).

The boundary halo is a per-sweep ReduceScatter: each core masks its edge
into the successor's chunk (host-provided mask), so the receive side is a
single contiguous DMA with no select/blend. LN/l2n are lean: h and h^2
packed in one bf16 tile summed by a single 6-matmul PSUM chain, rsqrt via
the 40k-entry Abs_reciprocal_sqrt ACT table (Newton fallback on the final
sweep), one f32r broadcast matmul, stride-0 broadcast APs for the apply,
and the shifted next-sweep input written directly by the LN apply. The
gate sigmoid is computed as (1+tanh(x/2))/2 so all F-sweep activations
(Gelu + Tanh) share one ACT table set - no per-sweep table reloads.

Startup: weight DMAs are chunked and alternated across both HWDGE queues
(sync/scalar) in first-use order; the PE warms up on embedding data while
they stream. lm_head: vocab-sharded bf16 weights DMA-preloaded during the
cheap tail into SBUF freed by the full-sweep weights; one bf16 AllGather
of final H, per-block gather-in DMAs.
"""
import sys, os
sys.path.insert(0, '/opt/trn_rl_repo')
import numpy as np
import ml_dtypes

import concourse.bass as bass
import concourse.bacc as bacc
import concourse.mybir as mybir
import concourse.tile as tile
from concourse.bass_utils import run_bass_kernel_spmd


def _ensure_ntff_hook():
    """The agent image's antenv lacks axon_hooks, which silently disables
    NTFF profiling (exec_time_ns). Shim the module and register the
    ctypes-based hook from trn_agent_boot if available."""
    import types
    if "antenv.axon_hooks" in sys.modules:
        return
    try:
        import antenv
        m = types.ModuleType("antenv.axon_hooks")
        _h = [None]
        m.set_axon_ntff_profile_hook = lambda h: _h.__setitem__(0, h)
        m.get_axon_ntff_profile_hook = lambda: _h[0]
        sys.modules["antenv.axon_hooks"] = m
        antenv.axon_hooks = m
        from trn_agent_boot.trn_boot import _ntff_profile_via_ctypes
        hook = _ntff_profile_via_ctypes("/opt/axon/libaxon_pjrt.so")
        if hook is not None:
            m.set_axon_ntff_profile_hook(hook)
    except Exception:
        pass


_ensure_ntff_hook()

F32 = mybir.dt.float32
F32R = mybir.dt.float32r
BF16 = mybir.dt.bfloat16
I32 = mybir.dt.int32
AF = mybir.ActivationFunctionType
ALU = mybir.AluOpType

N_CORES = 8
B, T, V, D, H = 4, 256, 50257, 768, 3072
ROWS = 128                 # rows per core = 32 timesteps x 4 batch
KC_D = D // 128            # 6 chunks of the d dimension
KC_H = H // 128            # 24 chunks of the hidden dimension
VPAD = 6400                # per-core vocab shard cols, padded to 50*128
VSHARD = 6283              # ceil(V / 8); host pads vocab to 8*VSHARD = 50264
SCHEDULE = "FFCCCCCFCCCCCCCC"
ALPHA = 0.4

LAST_RESULT = None         # BassKernelResults of the most recent run (for test.py)

_BUILD_CACHE = {}


def _t_layout(w):
    """[K, M] row-major -> [128, K/128, M] T-layout for stationary lhsT tiles."""
    K, M = w.shape
    assert K % 128 == 0
    return np.ascontiguousarray(w.reshape(K // 128, 128, M).transpose(1, 0, 2))


def _bcast_mid(ap2d, n):
    """[128, R] AP -> [128, n, R] stride-0 broadcast along the middle dim."""
    return bass.AP(ap2d.tensor, ap2d.offset, [ap2d.ap[0], (0, n), ap2d.ap[1]])


def build(schedule=SCHEDULE):
    n_sweeps = len(schedule)
    last_f = max(i for i, k in enumerate(schedule) if k == 'F')
    nc = bacc.Bacc("TRN2", target_bir_lowering=False, debug=False,
                   num_devices=N_CORES)

    # ---- DRAM parameters (per-core data via in_maps) ----
    embT_ext = nc.declare_dram_parameter("embT", [128, KC_D, ROWS], F32, isOutput=False)
    hmask_ext = nc.declare_dram_parameter("hmask", [128, 8, KC_D * 4], BF16,
                                          isOutput=False)
    wb_ext = {}
    for name, (wk, wm) in dict(Wgt=(D, D), V0=(D, H), V1=(H, D), W1=(D, D),
                               W2=(D, D), W2Wg=(D, D), RWt=(D, D), R=(D, D)).items():
        wb_ext[name] = nc.declare_dram_parameter(f"wb_{name}", [128, wk // 128, wm],
                                                 BF16, isOutput=False)
    wl_ext = nc.declare_dram_parameter("wl", [VPAD // 128, 128, KC_D, 128], BF16, isOutput=False)
    out_ext = nc.declare_dram_parameter("out", [VPAD, T * B], F32, isOutput=True)
    warm_ext = nc.declare_dram_parameter("warm", [128, 4], F32, isOutput=True)

    # ---- internal DRAM for collectives ----
    halo_in = [nc.dram_tensor(f"halo_in_{k}", [N_CORES * 128, KC_D * 4], BF16)
               for k in range(n_sweeps)]
    halo_out = [nc.dram_tensor(f"halo_out_{k}", [128, KC_D * 4], BF16)
                for k in range(n_sweeps)]
    ccw_in = nc.dram_tensor("ccw_in", [1, 32], F32)
    ccw_out = nc.dram_tensor("ccw_out", [N_CORES, 32], F32, addr_space="Shared")
    hfin_in = nc.dram_tensor("hfin_in", [128, KC_D * ROWS], BF16)
    hfin_out = nc.dram_tensor("hfin_out", [N_CORES * 128, KC_D * ROWS], BF16,
                              addr_space="Shared")

    rg = [list(range(N_CORES))]

    with tile.TileContext(nc) as tc:
        with (
            tc.tile_pool(name="cpool", bufs=1) as cpool,      # persistents
            tc.tile_pool(name="apool", bufs=1) as apool,      # activations
            tc.tile_pool(name="npool", bufs=1) as npool,      # norm scratch
            tc.tile_pool(name="pps", bufs=4, space="PSUM") as pps,
            tc.tile_pool(name="sps", bufs=2, space="PSUM") as sps,
        ):
            # ---------- persistent loads ----------
            embT = cpool.tile([128, KC_D, ROWS], F32, tag="embT")
            nc.sync.dma_start(embT[:], embT_ext[:])
            hmask = cpool.tile([128, 8, KC_D * 4], BF16, tag="hmask")
            nc.scalar.dma_start(hmask[:], hmask_ext[:])
            embTbf = cpool.tile([128, KC_D, ROWS], BF16, tag="embTbf")
            nc.vector.tensor_copy(embTbf[:], embT[:])
            rwt_t = cpool.tile([128, KC_D, D], BF16, tag="w_RWt")
            # warm up the collective path early (first call pays ENCD init)
            nc.sync.dma_start(ccw_in[:], embT[0:1, 0, 0:32])
            nc.gpsimd.collective_compute(
                "AllGather", ALU.bypass, replica_groups=rg,
                ins=[ccw_in[:]], outs=[ccw_out[:]])

            ones_col_bf = cpool.tile([128, 1], BF16, tag="ones_col_bf")
            nc.vector.memset(ones_col_bf[:], 1.0)
            ones_row_f = cpool.tile([1, 128], F32, tag="ones_row_f")
            nc.vector.memset(ones_row_f[:], 1.0)
            ones_row_r = cpool.tile([1, 128], F32R, tag="ones_row_r")
            nc.vector.tensor_copy(ones_row_r[:], ones_row_f[:])

            # persistent state
            Hs = [cpool.tile([128, KC_D, ROWS], BF16, tag=f"Hs{i}", name=f"Hs{i}")
                  for i in range(2)]
            EG = cpool.tile([128, KC_D, ROWS], F32, tag="EG")
            Psb = cpool.tile([128, KC_D, ROWS], F32, tag="Psb")    # frozen G*(CF-E)+E
            Gab = cpool.tile([128, KC_D, ROWS], BF16, tag="Gab")   # frozen alpha*G
            Hfbf = cpool.tile([128, KC_D, ROWS], BF16, tag="Hfbf")  # final H

            def mm_layer(w, Kc, Mc, rhs_fn, consume, group=4):
                wap = w if callable(w) else (
                    lambda kc, mc: w[:, kc, mc * 128:(mc + 1) * 128])
                for m0 in range(0, Mc, group):
                    g = min(group, Mc - m0)
                    p = pps.tile([128, g * 128], F32, tag="mmps")
                    for sub in range(g):
                        mc = m0 + sub
                        for kc in range(Kc):
                            nc.tensor.matmul(
                                p[:, sub * 128:(sub + 1) * 128],
                                wap(kc, mc),
                                rhs_fn(kc),
                                start=(kc == 0), stop=(kc == Kc - 1))
                    consume(p, m0, g)

            def nr_rsqrt_into(out_ap, s_ap, n, name, iters):
                """out_ap[1, n] = rsqrt(s_ap) via bit-trick seed + Newton."""
                bits = npool.tile([1, n], I32, tag=f"{name}b")
                nc.vector.tensor_scalar(bits[:], s_ap.bitcast(I32), 1, None,
                                        ALU.logical_shift_right)
                nc.vector.tensor_scalar(bits[:], bits[:], -1, 0x5f3759df,
                                        ALU.mult, ALU.add)
                cur = bits[:].bitcast(F32)
                for i in range(iters):
                    w = npool.tile([1, n], F32, tag=f"{name}w")
                    nc.vector.tensor_tensor(w[:], cur, cur, ALU.mult)
                    nc.vector.tensor_tensor(w[:], w[:], s_ap, ALU.mult)
                    nc.vector.tensor_scalar(w[:], w[:], -0.5, 1.5, ALU.mult, ALU.add)
                    if i == iters - 1:
                        nc.vector.tensor_tensor(out_ap, cur, w[:], ALU.mult)
                    else:
                        y = npool.tile([1, n], F32, tag=f"{name}y{i}")
                        nc.vector.tensor_tensor(y[:], cur, w[:], ALU.mult)
                        cur = y[:]

            def layer_norm_apply(hb2, it, nxt):
                """LN over hb2[:,:,0,:]; writes shifted nxt (or Hfbf if final)."""
                final = (it == n_sweeps - 1)
                nc.vector.tensor_tensor(hb2[:, 1], hb2[:, 0],
                                        hb2[:, 0], ALU.mult)
                ssum = sps.tile([1, 2 * ROWS], F32, tag="sum_ps")
                for kc in range(KC_D):
                    nc.tensor.matmul(ssum[:], ones_col_bf[:], hb2[:, :, kc, :],
                                     start=(kc == 0), stop=(kc == KC_D - 1))
                bc_in = npool.tile([1, 2 * ROWS], F32, tag="bc_in")
                nc.vector.tensor_scalar(bc_in[:, 0:ROWS], ssum[:, 0:ROWS],
                                        1.0 / D, None, ALU.mult)
                msq = npool.tile([1, ROWS], F32, tag="msq")
                nc.vector.tensor_tensor(msq[:], bc_in[:, 0:ROWS], bc_in[:, 0:ROWS],
                                        ALU.mult)
                var = npool.tile([1, ROWS], F32, tag="var")
                nc.vector.scalar_tensor_tensor(var[:], ssum[:, ROWS:2 * ROWS],
                                               1.0 / D, msq[:], ALU.mult,
                                               ALU.subtract)
                nc.vector.tensor_scalar(var[:], var[:], 1e-5, None, ALU.add)
                if final:
                    nr_rsqrt_into(bc_in[:, ROWS:2 * ROWS], var[:], ROWS, "ln",
                                  iters=2)
                else:
                    nc.scalar.activation(bc_in[:, ROWS:2 * ROWS], var[:],
                                         AF.Abs_reciprocal_sqrt)
                bcr = npool.tile([1, 2 * ROWS], F32R, tag="bcr")
                nc.vector.tensor_copy(bcr[:], bc_in[:])
                brp = pps.tile([128, 2 * ROWS], F32, tag="brp", bufs=2)
                nc.tensor.matmul(brp[:], ones_row_r[:], bcr[:], start=True, stop=True)
                dt_ = npool.tile([128, KC_D, ROWS], BF16, tag="dt")
                nc.vector.tensor_tensor(dt_[:], hb2[:, 0],
                                        _bcast_mid(brp[:, 0:ROWS], KC_D),
                                        ALU.subtract)
                if final:
                    nc.vector.tensor_tensor(Hfbf[:], dt_[:],
                                            _bcast_mid(brp[:, ROWS:2 * ROWS], KC_D),
                                            ALU.mult)
                    return None
                # shifted store: nxt rows 4.. <- own rows 0..123
                nc.vector.tensor_tensor(
                    nxt[:, :, 4:ROWS], dt_[:, :, 0:ROWS - 4],
                    _bcast_mid(brp[:, ROWS:2 * ROWS - 4], KC_D), ALU.mult)
                edge = apool.tile([128, KC_D, 4], BF16, tag="edge")
                nc.vector.tensor_tensor(
                    edge[:], dt_[:, :, ROWS - 4:ROWS],
                    _bcast_mid(brp[:, 2 * ROWS - 4:2 * ROWS], KC_D), ALU.mult)
                return edge

            def sweep_tail(it, nxt, edge):
                """Launch this sweep's halo; consume the lag-3 halo into nxt.

                The halo is a ReduceScatter: each core contributes its edge
                masked into the successor core's chunk (hmask, host data), so
                the receive side is a single contiguous DMA with no blend."""
                if it <= n_sweeps - 3:
                    masked = npool.tile([128, 8, KC_D * 4], BF16, tag="masked")
                    eflat = edge[:].rearrange("p k c -> p (k c)")
                    nc.vector.tensor_tensor(masked[:], _bcast_mid(eflat, 8),
                                            hmask[:], ALU.mult)
                    half_rows = 4 * 128
                    nc.sync.dma_start(
                        halo_in[it].ap()[0:half_rows, :].rearrange(
                            "(j p) f -> p j f", p=128),
                        masked[:, 0:4, :])
                    nc.scalar.dma_start(
                        halo_in[it].ap()[half_rows:2 * half_rows, :].rearrange(
                            "(j p) f -> p j f", p=128),
                        masked[:, 4:8, :])
                    nc.gpsimd.collective_compute(
                        "ReduceScatter", ALU.add, replica_groups=rg,
                        ins=[halo_in[it][:]], outs=[halo_out[it][:]])
                if it >= 2:
                    hstage = npool.tile([128, KC_D * 4], BF16, tag="hstage")
                    nc.sync.dma_start(hstage[:], halo_out[it - 2][:])
                    nc.vector.tensor_copy(
                        nxt[:, :, 0:4],
                        hstage[:].rearrange("p (k c) -> p k c", k=KC_D))
                else:
                    nc.vector.memset(nxt[:, :, 0:4], 0.0)

            def hp_consume_fn(hb2):
                def f(p, m0, g):
                    t2 = apool.tile([128, g * 128], F32, tag=f"t2_{m0 % 8}")
                    nc.vector.tensor_tensor(t2[:], p[:], Gab[:, m0:m0 + g, :],
                                            ALU.mult)
                    nc.vector.tensor_tensor(hb2[:, 0, m0:m0 + g, :], t2[:],
                                            Psb[:, m0:m0 + g, :], ALU.add)
                return f

            def full_sweep(it, wsb, cur, nxt):
                first = (it == 0)
                if first:
                    CTX = embTbf
                else:
                    CTX = apool.tile([128, KC_D, ROWS], BF16, tag="CTX", bufs=2)

                    def ctx_consume(p, m0, g):
                        nc.vector.scalar_tensor_tensor(
                            CTX[:, m0:m0 + g, :], p[:], ALPHA,
                            embT[:, m0:m0 + g, :], ALU.mult, ALU.add)
                    mm_layer(wsb["R"], KC_D, KC_D, lambda kc: cur[:, kc, :], ctx_consume)

                Abf = apool.tile([128, KC_H, ROWS], BF16, tag="Abf")

                def gelu_consume(dst):
                    def f(p, m0, g):
                        nc.scalar.activation(dst[:, m0:m0 + g, :], p[:], AF.Gelu)
                    return f
                mm_layer(wsb["V0"], KC_D, KC_H, lambda kc: CTX[:, kc, :],
                         gelu_consume(Abf))
                TGTbf = apool.tile([128, KC_D, ROWS], BF16, tag="TGTbf", bufs=2)
                mm_layer(wsb["V1"], KC_H, KC_D, lambda kc: Abf[:, kc, :],
                         gelu_consume(TGTbf))

                # TF matmuls early: fill the PE gap while the l2n chain runs
                tf_ps = []
                if not first:
                    mm_layer(rwt_t, KC_D, KC_D, lambda kc: cur[:, kc, :],
                             lambda p, m0, g: tf_ps.append((p, m0, g)))

                # CB = l2n(TGT)
                sq = npool.tile([128, KC_D, ROWS], BF16, tag="sq")
                nc.vector.tensor_tensor(sq[:], TGTbf[:], TGTbf[:], ALU.mult)
                ssp = sps.tile([1, 2 * ROWS], F32, tag="sum_ps")
                for kc in range(KC_D):
                    nc.tensor.matmul(ssp[:, 0:ROWS], ones_col_bf[:], sq[:, kc, :],
                                     start=(kc == 0), stop=(kc == KC_D - 1))
                ss = npool.tile([1, ROWS], F32, tag="ss")
                nc.vector.tensor_scalar(ss[:], ssp[:, 0:ROWS], 1e-24, None, ALU.add)
                rl2 = npool.tile([1, ROWS], F32, tag="rl2")
                nr_rsqrt_into(rl2[:], ss[:], ROWS, "l2n", iters=2)
                rl2r = npool.tile([1, ROWS], F32R, tag="rl2r")
                nc.vector.tensor_copy(rl2r[:], rl2[:])
                rbp = pps.tile([128, ROWS], F32, tag="brp", bufs=2)
                nc.tensor.matmul(rbp[:], ones_row_r[:], rl2r[:], start=True, stop=True)
                CBbf = apool.tile([128, KC_D, ROWS], BF16, tag="CBbf")
                nc.vector.tensor_tensor(CBbf[:], TGTbf[:], _bcast_mid(rbp[:], KC_D),
                                        ALU.mult)

                Ubf = apool.tile([128, KC_D, ROWS], BF16, tag="Ubf", bufs=2)
                mm_layer(wsb["W1"], KC_D, KC_D, lambda kc: CBbf[:, kc, :],
                         gelu_consume(Ubf))

                if first:
                    # EG = embT @ Wg_top, emitted here so its matmuls sit
                    # behind F0's V0/V1 work in PE program order (Wgt's DMA
                    # arrives later than V0's).
                    def eg_consume(p, m0, g):
                        nc.vector.tensor_copy(EG[:, m0:m0 + g, :], p[:])
                    mm_layer(wsb["Wgt"], KC_D, KC_D, lambda kc: embTbf[:, kc, :],
                             eg_consume)

                CFbf = apool.tile([128, KC_D, ROWS], BF16, tag="CFbf", bufs=2)

                def cf_consume(p, m0, g):
                    nc.scalar.copy(CFbf[:, m0:m0 + g, :], p[:])
                mm_layer(wsb["W2"], KC_D, KC_D, lambda kc: Ubf[:, kc, :], cf_consume)

                # gate via tanh (shares the Gelu ACT table):
                # G = sigmoid(x) = (1+tanh(x/2))/2; store th = tanh(x/2)
                Gsb = apool.tile([128, KC_D, ROWS], BF16, tag="Gsb")

                def g_consume(p, m0, g):
                    gin = apool.tile([128, g * 128], F32, tag=f"gin{m0 % 8}")
                    nc.vector.tensor_tensor(gin[:], p[:], EG[:, m0:m0 + g, :], ALU.add)
                    nc.scalar.activation(Gsb[:, m0:m0 + g, :], gin[:], AF.Tanh,
                                         scale=0.5)
                mm_layer(wsb["W2Wg"], KC_D, KC_D, lambda kc: Ubf[:, kc, :], g_consume)

                # Ga = alpha*(1+th)/2 ; P = ((1+th)/2)*(CF-E)+E
                nc.vector.tensor_scalar(Gab[:], Gsb[:], ALPHA / 2, ALPHA / 2,
                                        ALU.mult, ALU.add)
                t_ = apool.tile([128, KC_D, ROWS], F32, tag="pt")
                nc.vector.tensor_tensor(t_[:], CFbf[:], embT[:], ALU.subtract)
                nc.vector.scalar_tensor_tensor(t_[:], Gsb[:], 1.0, t_[:],
                                               ALU.add, ALU.mult)
                nc.vector.scalar_tensor_tensor(Psb[:], t_[:], 0.5, embT[:],
                                               ALU.mult, ALU.add)

                hb2 = npool.tile([128, 2, KC_D, ROWS], BF16, tag="hb2")
                if first:
                    nc.vector.tensor_copy(hb2[:, 0], Psb[:])
                else:
                    hpc = hp_consume_fn(hb2)
                    for (p, m0, g) in tf_ps:
                        hpc(p, m0, g)
                return layer_norm_apply(hb2, it, nxt)

            def cheap_sweep(it, cur, nxt):
                hb2 = npool.tile([128, 2, KC_D, ROWS], BF16, tag="hb2")
                mm_layer(rwt_t, KC_D, KC_D, lambda kc: cur[:, kc, :],
                         hp_consume_fn(hb2))
                return layer_norm_apply(hb2, it, nxt)

            # ---------- phase 1: sweeps up to and including the last F ----------
            with tc.tile_pool(name="fwpool", bufs=1) as fwpool:
                # PE warm-up on data available immediately: ramps the PE clock
                # while the weight DMAs stream in.
                wu_ps = pps.tile([128, 512], F32, tag="mmps")
                for i in range(12):
                    nc.tensor.matmul(wu_ps[:], embTbf[:, 0, 0:128],
                                     embTbf[:, (i % 3):(i % 3) + 4, :],
                                     start=(i == 0), stop=(i == 11))
                wu_sb = cpool.tile([128, 4], F32, tag="wu_sb")
                nc.vector.tensor_copy(wu_sb[:], wu_ps[:, 0:4])
                nc.sync.dma_start(warm_ext[:], wu_sb[:])

                # weight loads: chunked and alternated across the two DMA
                # engines, ordered by first use so F0 can start immediately.
                qi = [0]

                def qdma(dst_ap, src_ap):
                    eng = nc.sync if qi[0] % 2 == 0 else nc.scalar
                    qi[0] += 1
                    eng.dma_start(dst_ap, src_ap)

                v0_t = [fwpool.tile([128, H], BF16, tag=f"w_V0_{kc}", name=f"w_V0_{kc}")
                        for kc in range(KC_D)]
                for kc in range(KC_D):
                    qdma(v0_t[kc][:], wb_ext["V0"][:, kc, :])
                v1_t = [fwpool.tile([128, 6, D], BF16, tag=f"w_V1_{b}", name=f"w_V1_{b}")
                        for b in range(4)]
                for b_ in range(4):
                    qdma(v1_t[b_][:], wb_ext["V1"][:, b_ * 6:(b_ + 1) * 6, :])
                wsb = {"V0": lambda kc, mc: v0_t[kc][:, mc * 128:(mc + 1) * 128],
                       "V1": lambda kc, mc: v1_t[kc // 6][:, kc % 6,
                                                          mc * 128:(mc + 1) * 128]}
                for name in ("W1", "W2", "W2Wg", "Wgt", "R"):
                    ext = wb_ext[name]
                    t_ = fwpool.tile(list(ext.shape), BF16, tag=f"w_{name}")
                    qdma(t_[:], ext[:])
                    wsb[name] = t_
                qdma(rwt_t[:], wb_ext["RWt"][:])

                for it in range(last_f + 1):
                    cur = Hs[it % 2]
                    nxt = Hs[(it + 1) % 2]
                    if schedule[it] == 'F':
                        edge = full_sweep(it, wsb, cur, nxt)
                    else:
                        edge = cheap_sweep(it, cur, nxt)
                    sweep_tail(it, nxt, edge)

            # ---------- phase 2: cheap tail + lm_head ----------
            with (
                tc.tile_pool(name="wlpool", bufs=1) as wlpool,
                tc.tile_pool(name="opool", bufs=4) as opool,
            ):
                NV = VPAD // 128
                wl_t = wlpool.tile([128, NV, KC_D, 128], BF16, tag="wl")
                for vc in range(NV):
                    nc.scalar.dma_start(wl_t[:, vc], wl_ext[vc])

                for it in range(last_f + 1, n_sweeps):
                    cur = Hs[it % 2]
                    nxt = Hs[(it + 1) % 2]
                    edge = cheap_sweep(it, cur, nxt)
                    if it + 1 < n_sweeps:
                        sweep_tail(it, nxt, edge)

                # final H gather (bf16)
                nc.sync.dma_start(
                    hfin_in.ap().rearrange("p (k c) -> p k c", k=KC_D), Hfbf[:])
                nc.gpsimd.collective_compute(
                    "AllGather", ALU.bypass, replica_groups=rg,
                    ins=[hfin_in[:]], outs=[hfin_out[:]])
                Hfull = cpool.tile([128, KC_D, T * B], BF16, tag="Hfull")
                for r in range(N_CORES):
                    eng = nc.sync if r % 2 == 0 else nc.scalar
                    eng.dma_start(
                        Hfull[:, :, r * 128:(r + 1) * 128],
                        hfin_out.ap()[r * 128:(r + 1) * 128, :].rearrange(
                            "p (k c) -> p k c", k=KC_D))

                for vc in range(NV):
                    for half in range(2):
                        p = pps.tile([128, 512], F32, tag="mmps")
                        for kc in range(KC_D):
                            nc.tensor.matmul(
                                p[:], wl_t[:, vc, kc, :],
                                Hfull[:, kc, half * 512:(half + 1) * 512],
                                start=(kc == 0), stop=(kc == KC_D - 1))
                        osb = opool.tile([128, 512], F32, tag="osb")
                        if half == 0:
                            nc.vector.tensor_copy(osb[:], p[:])
                        else:
                            nc.scalar.copy(osb[:], p[:])
                        eng = nc.sync if half == 0 else nc.scalar
                        eng.dma_start(
                            out_ext[vc * 128:(vc + 1) * 128,
                                    half * 512:(half + 1) * 512], osb[:])

    nc.compile()
    return nc


def _get_built(schedule=SCHEDULE):
    if schedule not in _BUILD_CACHE:
        _BUILD_CACHE[schedule] = build(schedule)
    return _BUILD_CACHE[schedule]


def _prep_in_maps(token_ids, embedding, V0, b0, V1, b1, W1, c1, W2, c2, Wg, bg,
                  Wt, gamma, beta, Wl, R_weight):
    f64 = np.float64
    for z in (b0, b1, c1, c2, bg, beta):
        assert np.count_nonzero(np.asarray(z)) == 0, "nonzero bias unsupported"
    assert np.allclose(np.asarray(gamma), 1.0), "gamma != 1 unsupported"

    tok = np.asarray(token_ids).astype(np.int64)           # [B, T]
    emb = np.asarray(embedding, f64)[tok]                  # [B, T, D]
    emb = emb / np.maximum(np.linalg.norm(emb, axis=-1, keepdims=True), 1e-12)
    rows = emb.transpose(1, 0, 2).reshape(T * B, D)        # row = t*4+b

    bf = ml_dtypes.bfloat16
    wt = {
        "R": _t_layout(np.asarray(R_weight, f64)).astype(bf),
        "V0": _t_layout(np.asarray(V0, f64)).astype(bf),
        "V1": _t_layout(np.asarray(V1, f64)).astype(bf),
        "W1": _t_layout(np.asarray(W1, f64)).astype(bf),
        "W2": _t_layout(np.asarray(W2, f64)).astype(bf),
        "RWt": _t_layout(np.asarray(R_weight, f64) @ np.asarray(Wt, f64)).astype(bf),
        "Wgt": _t_layout(np.asarray(Wg, f64)[:D]).astype(bf),
        "W2Wg": _t_layout(np.asarray(W2, f64) @ np.asarray(Wg, f64)[D:]).astype(bf),
    }
    wl_f32 = np.asarray(Wl, np.float32)

    in_maps = []
    for c in range(N_CORES):
        block = rows[c * ROWS:(c + 1) * ROWS].T            # [D, 128]
        embT = np.ascontiguousarray(
            block.reshape(KC_D, 128, ROWS).transpose(1, 0, 2)).astype(np.float32)
        hm = np.zeros((8, KC_D * 4), np.float32)
        if c < N_CORES - 1:
            hm[c + 1, :] = 1.0
        hmask = np.broadcast_to(hm, (128, 8, KC_D * 4)).astype(bf)
        wl_shard_cols = np.zeros((D, VPAD), np.float32)
        lo = c * VSHARD
        hi = min(V, lo + VSHARD)
        wl_shard_cols[:, :hi - lo] = wl_f32[:, lo:hi]
        wl_shard = _t_layout(wl_shard_cols)                 # [128, KC_D, VPAD]
        wl_shard = np.ascontiguousarray(
            wl_shard.reshape(128, KC_D, VPAD // 128, 128).transpose(2, 0, 1, 3)).astype(bf)
        m = {"embT": embT, "hmask": np.ascontiguousarray(hmask), "wl": wl_shard}
        for name, w in wt.items():
            m[f"wb_{name}"] = w
        in_maps.append(m)
    return in_maps


def kernel(**inputs):
    global LAST_RESULT
    in_maps = _prep_in_maps(**{k: np.asarray(v) for k, v in inputs.items()})
    nc = _get_built()
    trace = bool(os.environ.get("KERNEL_TRACE"))
    res = run_bass_kernel_spmd(nc, in_maps, core_ids=list(range(N_CORES)),
                               trace=trace)
    LAST_RESULT = res
    parts = [res.results[c]["out"][:VSHARD] for c in range(N_CORES)]
    L = np.concatenate(parts, axis=0)[:V]                  # [V, T*B]
    out = np.ascontiguousarray(
        L.reshape(V, T, B).transpose(2, 1, 0)).astype(np.float32)
    return out


if __name__ == "__main__":
    pass
